# revision 22
# baseline (speedup 1.0000x reference)
"""nn_AttentionBlock Trainium2 Bass kernel (fp8 DoubleRow edition).

kernel(**inputs): FULL unsharded inputs (x [8,512,32,32], gamma/beta [512],
w_qkv [1536,512], b_qkv [1536], w_proj [512,512], b_proj [512]) -> FULL
output [8,512,32,32] float32.

Sharding: data-parallel over batch, one batch element per NeuronCore (8 cores),
no collectives. Per core:
  GroupNorm: per-channel (sum, sumsq) stats fully on DVE (reduce_sum +
  tensor_tensor_reduce), group-combined with tiny mask matmuls;
  rstd = exp(-0.5*ln(var+eps)) keeps ACT on the ln/exp table set.
  xn stored fp8e4; qkv / v^T / proj matmuls run fp8 DoubleRow (2 contraction
  rows per partition, 2x PE throughput); w_proj is pre-scaled by 512 so its
  entries are representable in fp8, compensated by 2^-9 in the residual
  combine; b_eff enters the proj psum via a rank-1 ones matmul.
  Scores St[t,s] per head stay f32r from f32r q/k copies; exp runs on ACT
  with scale=1/8, bias=-2 (keeps fp8e4 output < 240) writing paired tiles
  [128, 2, S] that feed the fp8 DoubleRow AV matmul directly; softmax
  denominator folded in as an extra ones-column of the packed lhsT.
  Division deferred: reciprocal of the r-row, DRAM-bounce row broadcast,
  A = av * rinv (fp8) feeds proj.
"""
import sys
sys.path.insert(0, '/opt/trn_rl_repo')
import numpy as np
import concourse.bacc as bacc
import concourse.mybir as mybir
import concourse.tile as tile

F32 = mybir.dt.float32
F32R = mybir.dt.float32r
F8 = mybir.dt.float8e4
AF = mybir.ActivationFunctionType
DR = mybir.MatmulPerfMode.DoubleRow
MUL = mybir.AluOpType.mult
ADD = mybir.AluOpType.add

C, S, NH, HD, G = 512, 1024, 8, 64, 32
KS = 4          # k-subtiles of 128 over C
SC = 2          # s-chunks of 512
TC = 8          # t-chunks of 128
OC = 4          # output channel chunks of 128
EPS = 1e-5
PSCL = 2.0 ** -9   # proj weights are prepped x512; undo here


def emit_x_load(nc, pools, dram):
    x_sb = pools["xp"].tile([128, OC, S], F32, tag="x")
    xr = dram["x"].ap().rearrange("(j p) s -> p j s", p=128)
    for j in range(OC):
        for half in range(2):
            nc.sync.dma_start(x_sb[:, j, half * 512:(half + 1) * 512],
                              xr[:, j, half * 512:(half + 1) * 512])
    return x_sb


def emit_body(nc, tc_ctx, pools, dram, p_const, x_sb=None):
    (x_d, y_d) = dram["x"], dram["y"]
    sb1 = pools["sb1"]
    qkp = pools["qk"]
    exp_p = pools["exp"]
    outp = pools["out"]
    rp = pools["recip"]
    ps_big = pools["ps_big"]   # [128,1024] psum (2 banks/slot)
    ps_sm = pools["ps_sm"]     # [128,512] psum (1 bank/slot)
    dbg = dram.get("dbg")
    av_lhs = p_const["av_lhs"]

    # ---------------- GroupNorm ----------------
    if x_sb is None:
        x_sb = emit_x_load(nc, pools, dram)

    # warm the ACT tables (Ln/Exp) while DMAs are in flight
    warm = sb1.tile([32, 1], F32, tag="warm")
    nc.scalar.activation(out=warm[:], in_=p_const["eps"][0:32, :], func=AF.Exp)
    nc.scalar.activation(out=warm[:], in_=warm[:], func=AF.Ln)

    # per-channel stats via bn_stats on 512-halves (even/odd sub-stats each):
    # four quarter-stats (count=256, mean m_q, 256*var cv_q) per channel.
    # stats2 = [sum_q m_q, sum_q cv_q + 256*sum_q m_q^2]; the /16 mask matmul
    # plus INV0=0.25 / INV1=1/1024 recover group mean and E[x^2].
    bs6 = sb1.tile([128, OC, 2, 6], F32, tag="bs6")
    stats2 = sb1.tile([128, OC, 2], F32, tag="stats2")
    scr = sb1.tile([128, OC, 2, 4], F32, tag="scr")
    for j in range(OC):
        for hl in range(2):
            nc.vector.bn_stats(bs6[:, j, hl, :], x_sb[:, j, hl * 512:(hl + 1) * 512])
    m0 = bs6[:, :, 0, 1:5:3]   # [128, OC, 2] = (m_e, m_o) of half 0
    m1 = bs6[:, :, 1, 1:5:3]
    cv0 = bs6[:, :, 0, 2:6:3]
    cv1 = bs6[:, :, 1, 2:6:3]
    nc.vector.tensor_tensor(scr[:, :, :, 0], m0, m1, ADD)
    nc.vector.tensor_tensor(stats2[:, :, 0], scr[:, :, 0, 0], scr[:, :, 1, 0], ADD)
    nc.vector.tensor_mul(scr[:, :, :, 1], m0, m0)
    nc.vector.tensor_mul(scr[:, :, :, 2], m1, m1)
    nc.vector.tensor_tensor(scr[:, :, :, 1], scr[:, :, :, 1], scr[:, :, :, 2], ADD)
    nc.vector.tensor_tensor(scr[:, :, 0, 1], scr[:, :, 0, 1], scr[:, :, 1, 1], ADD)
    nc.vector.tensor_tensor(scr[:, :, :, 3], cv0, cv1, ADD)
    nc.vector.tensor_tensor(scr[:, :, 0, 3], scr[:, :, 0, 3], scr[:, :, 1, 3], ADD)
    nc.vector.scalar_tensor_tensor(out=stats2[:, :, 1], in0=scr[:, :, 0, 1],
                                   scalar=256.0, in1=scr[:, :, 0, 3],
                                   op0=MUL, op1=ADD)

    # group combine: gstats[g, st] = sum_{c in g} stats2[c, st] / 16
    gs_ps = ps_sm.tile([32, 2], F32, tag="ps_sm")
    for j in range(OC):
        nc.tensor.matmul(gs_ps[:], p_const["gn_fwd"][:, j, :], stats2[:, j, :],
                         start=(j == 0), stop=(j == OC - 1))
    gs_sb = sb1.tile([32, 2], F32, tag="gs_sb")
    nc.vector.tensor_copy(gs_sb[:], gs_ps[:])
    gm = sb1.tile([32, 2], F32, tag="gm")          # (M_g, rstd_g)
    gv = sb1.tile([32, 1], F32, tag="gv")
    nc.vector.tensor_scalar_mul(gm[:, 0:1], gs_sb[:, 0:1], 0.25)
    nc.vector.tensor_scalar_mul(gv[:], gs_sb[:, 1:2], 1.0 / 1024.0)
    msq = sb1.tile([32, 1], F32, tag="msq")
    nc.vector.tensor_mul(msq[:], gm[:, 0:1], gm[:, 0:1])
    nc.vector.tensor_tensor(gv[:], gv[:], msq[:], mybir.AluOpType.subtract)
    # rstd = exp(-0.5*ln(var+eps)) — stays on the ln/exp ACT table set
    lnv = sb1.tile([32, 1], F32, tag="lnv")
    nc.scalar.activation(out=lnv[:], in_=gv[:], func=AF.Ln, bias=p_const["eps"][0:32, :], scale=1.0)
    nc.scalar.activation(out=gm[:, 1:2], in_=lnv[:], func=AF.Exp, scale=-0.5)

    # broadcast back per channel chunk: means to cols [0:OC], rstds to [OC:2OC]
    mb_ps = ps_sm.tile([128, 2 * OC], F32, tag="ps_sm")
    for j in range(OC):
        nc.tensor.matmul(mb_ps[:, j:j + 1], p_const["gn_bwd"][:, j, :], gm[:, 0:1],
                         start=True, stop=True)
        nc.tensor.matmul(mb_ps[:, OC + j:OC + j + 1], p_const["gn_bwd"][:, j, :], gm[:, 1:2],
                         start=True, stop=True)
    Acol = sb1.tile([128, OC], F32, tag="Acol")
    Bcol = sb1.tile([128, OC], F32, tag="Bcol")
    nc.vector.tensor_mul(Acol[:], mb_ps[:, OC:2 * OC], p_const["gamma"][:])
    nc.vector.tensor_mul(Bcol[:], mb_ps[:, 0:OC], Acol[:])
    nc.vector.tensor_tensor(Bcol[:], p_const["beta"][:], Bcol[:], mybir.AluOpType.subtract)
    # xn = x * Acol + Bcol, stored fp8e4 (on the otherwise-idle Pool engine)
    xn_sb = sb1.tile([128, KS, S], F8, tag="xn")
    for j in range(OC):
        nc.gpsimd.tensor_scalar(out=xn_sb[:, j, :], in0=x_sb[:, j, :],
                                scalar1=Acol[:, j:j + 1], scalar2=Bcol[:, j:j + 1],
                                op0=MUL, op1=ADD)

    if dbg is not None:
        nc.gpsimd.dma_start(dbg["xn"].ap().rearrange("(j p) s -> p j s", p=128), xn_sb[:])

    # ---------------- emit helpers ----------------
    qk_sb = {}

    def emit_qkv_chunk(h):
        # q/k head-pair chunk h: fp8 DoubleRow over ks pairs -> f32r + bias
        qk_t = qkp.tile([128, S], F32R, tag="qk", name=f"qk{h}")
        for sc in range(SC):
            qkv_ps = ps_sm.tile([128, 512], F32, tag="ps_sm", name=f"qkvps{h}_{sc}")
            for jp in range(2):
                nc.tensor.matmul(qkv_ps[:],
                                 p_const["wqk"][:, 2 * jp:2 * jp + 2, h, :],
                                 xn_sb[:, 2 * jp:2 * jp + 2, sc * 512:(sc + 1) * 512],
                                 start=(jp == 0), stop=(jp == 1), perf_mode=DR)
            nc.vector.tensor_scalar_add(qk_t[:, sc * 512:(sc + 1) * 512], qkv_ps[:],
                                        p_const["bqk"][:, h:h + 1])
        qk_sb[h] = qk_t

    def emit_vt():
        # v^T tiles: fp8 DoubleRow, repacked into av_lhs fp8 slots
        for t in range(TC):
            vt_ps = ps_sm.tile([128, 512], F32, tag="ps_sm", name=f"vtps{t}")
            for jp in range(2):
                nc.tensor.matmul(vt_ps[:],
                                 xn_sb[:, 2 * jp:2 * jp + 2, t * 128:(t + 1) * 128],
                                 p_const["wvt"][:, 2 * jp:2 * jp + 2, :],
                                 start=(jp == 0), stop=(jp == 1), perf_mode=DR)
            src = vt_ps[:].rearrange("p (pr two m) -> p pr two m", two=2, m=64)
            dst = av_lhs[:, t].rearrange("p (pr two) m -> p pr two m", two=2)
            nc.vector.tensor_copy(dst[:, :, 0, 0:64], src[:, :, 0, :])
            nc.vector.tensor_copy(dst[:, :, 1, 64:128], src[:, :, 1, :])

    A_sb = pools["ap2"].tile([128, KS, S], F8, tag="A")

    def emit_scores_exp(h):
        # St[t,s] (f32r) then exp -> fp8 pair tiles [128, 2, S]
        p, half = h // 2, h % 2
        qq, kk = qk_sb[2 * p], qk_sb[2 * p + 1]
        lo = slice(64 * half, 64 * half + 64)
        pairs = []
        for u in range(TC // 2):
            ep = exp_p.tile([128, 2, S], F8, tag="expst", name=f"e{h}_{u}")
            pairs.append(ep)
        for t in range(TC):
            st0 = ps_big.tile([128, S], F32, tag="ps_big", name=f"st{h}_{t}")
            for sc in range(SC):
                nc.tensor.matmul(st0[:, sc * 512:(sc + 1) * 512],
                                 kk[lo, t * 128:(t + 1) * 128],
                                 qq[lo, sc * 512:(sc + 1) * 512],
                                 start=True, stop=True)
            nc.scalar.activation(out=pairs[t // 2][:, t % 2, :], in_=st0[:],
                                 func=AF.Exp, scale=0.125, bias=p_const["negtwo"][:, :])
        return pairs

    def emit_av(h, pairs):
        p, half = h // 2, h % 2
        av_ps = ps_big.tile([128, S], F32, tag="ps_big", name=f"av{h}")
        a_sl = slice(0, 64) if half == 0 else slice(64, 128)
        r_row = slice(64, 65) if half == 0 else slice(0, 1)
        for sc in range(SC):
            for u in range(TC // 2):
                nc.tensor.matmul(av_ps[:, sc * 512:(sc + 1) * 512],
                                 av_lhs[:, 2 * u:2 * u + 2, h, :],
                                 pairs[u][:, :, sc * 512:(sc + 1) * 512],
                                 start=(u == 0), stop=(u == TC // 2 - 1),
                                 perf_mode=DR)
        # rinv on the single r partition row; DRAM-bounce broadcast to a_sl rows
        # (SBUF APs reject partition-stride 0; DVE can't read 2 PSUM inputs)
        import concourse.bass as bass
        rstage = rp.tile([128, S], F32, tag="rstage", name=f"rs{h}")
        nc.vector.reciprocal(out=rstage[r_row, :], in_=av_ps[r_row, :])
        nc.sync.dma_start(dram["r_scr"].ap()[h:h + 1, :], rstage[r_row, :])
        rb = rp.tile([128, S], F32, tag="rb", name=f"rb{h}")
        bcast_src = bass.AP(tensor=dram["r_scr"], offset=h * S, ap=[[0, 64], [1, S]])
        nc.gpsimd.dma_start(rb[a_sl, :], bcast_src)
        nc.vector.tensor_tensor(A_sb[a_sl, p, :], av_ps[a_sl, :], rb[a_sl, :], MUL)

    o_ts = [outp.tile([128, S], F32, tag="o", name=f"o{oc}") for oc in range(OC)]

    def emit_proj_half(hf):
        # o = x + 2^-9 * (b_eff*512 + (w_proj*512) @ A)
        for oc in range(OC):
            for sc in range(SC):
                pj_ps = ps_sm.tile([128, 512], F32, tag="ps_sm", name=f"pj{hf}_{oc}_{sc}")
                if hf == 0:
                    nc.tensor.matmul(pj_ps[:], p_const["beff"][0:1, oc, :],
                                     p_const["ones8"][0:1, :],
                                     start=True, stop=False)
                nc.tensor.matmul(pj_ps[:],
                                 p_const["wpt"][:, 2 * hf:2 * hf + 2, oc * 128:(oc + 1) * 128],
                                 A_sb[:, 2 * hf:2 * hf + 2, sc * 512:(sc + 1) * 512],
                                 start=False if hf == 0 else True, stop=True,
                                 perf_mode=DR)
                base = x_sb[:, oc, sc * 512:(sc + 1) * 512] if hf == 0 else \
                    o_ts[oc][:, sc * 512:(sc + 1) * 512]
                nc.vector.scalar_tensor_tensor(
                    out=o_ts[oc][:, sc * 512:(sc + 1) * 512],
                    in0=pj_ps[:], scalar=PSCL, in1=base, op0=MUL, op1=ADD)
            if hf == 1:
                nc.sync.dma_start(y_d.ap().rearrange("(j p) s -> p j s", p=128)[:, oc, :],
                                  o_ts[oc][:])

    # ---------------- pipelined schedule ----------------
    emit_qkv_chunk(0)
    emit_qkv_chunk(1)
    et0 = emit_scores_exp(0)
    emit_qkv_chunk(2)
    emit_qkv_chunk(3)
    et1 = emit_scores_exp(1)
    emit_vt()
    emit_av(0, et0)
    emit_qkv_chunk(4)
    emit_qkv_chunk(5)
    et2 = emit_scores_exp(2)
    emit_av(1, et1)
    emit_qkv_chunk(6)
    emit_qkv_chunk(7)
    et3 = emit_scores_exp(3)
    emit_av(2, et2)
    et4 = emit_scores_exp(4)
    emit_av(3, et3)
    emit_proj_half(0)
    et5 = emit_scores_exp(5)
    emit_av(4, et4)
    et6 = emit_scores_exp(6)
    emit_av(5, et5)
    et7 = emit_scores_exp(7)
    emit_av(6, et6)
    emit_av(7, et7)
    emit_proj_half(1)

    if dbg is not None:
        nc.gpsimd.dma_start(dbg["A"].ap().rearrange("(j p) s -> p j s", p=128), A_sb[:])


def build_nc(loop_iters=1, debug_outputs=False):
    nc = bacc.Bacc(None, target_bir_lowering=False)
    dram = {
        "x": nc.dram_tensor("x", [C, S], F32, kind="ExternalInput"),
        "y": nc.dram_tensor("y", [C, S], F32, kind="ExternalOutput"),
        "r_scr": nc.dram_tensor("r_scr", [NH, S], F32),
    }
    w_in = {
        "wqk": nc.dram_tensor("wqk", [KS, 128, NH, 128], F32, kind="ExternalInput"),
        "wvt": nc.dram_tensor("wvt", [KS, 128, 512], F32, kind="ExternalInput"),
        "wpt": nc.dram_tensor("wpt", [KS, 128, 512], F32, kind="ExternalInput"),
        "bqk": nc.dram_tensor("bqk", [128, NH], F32, kind="ExternalInput"),
        "beff": nc.dram_tensor("beff", [1, OC, 128], F32, kind="ExternalInput"),
        "gamma": nc.dram_tensor("gamma", [128, OC], F32, kind="ExternalInput"),
        "beta": nc.dram_tensor("beta", [128, OC], F32, kind="ExternalInput"),
        "gn_fwd": nc.dram_tensor("gn_fwd", [OC, 128, 32], F32, kind="ExternalInput"),
        "gn_bwd": nc.dram_tensor("gn_bwd", [OC, 32, 128], F32, kind="ExternalInput"),
    }
    if debug_outputs:
        dram["dbg"] = {
            "xn": nc.dram_tensor("dbg_xn", [C, S], F32, kind="ExternalOutput"),
            "A": nc.dram_tensor("dbg_A", [C, S], F32, kind="ExternalOutput"),
        }

    with tile.TileContext(nc) as tctx:
        with (
            tctx.tile_pool(name="const", bufs=1) as cp,
            tctx.tile_pool(name="sb1", bufs=1) as sb1,
            tctx.tile_pool(name="xp", bufs=2) as xp,
            tctx.tile_pool(name="ap2", bufs=2) as ap2,
            tctx.tile_pool(name="qk", bufs=NH) as qkp,
            tctx.tile_pool(name="exp", bufs=8) as exp_p,
            tctx.tile_pool(name="out", bufs=4) as outp,
            tctx.tile_pool(name="recip", bufs=2) as rp,
            tctx.tile_pool(name="ps_big", bufs=3, space="PSUM") as ps_big,
            tctx.tile_pool(name="ps_sm", bufs=2, space="PSUM") as ps_sm,
        ):
            pools = dict(sb1=sb1, xp=xp, ap2=ap2, qk=qkp, exp=exp_p, out=outp,
                         recip=rp, ps_big=ps_big, ps_sm=ps_sm)

            # tiny consts first (masks gate the GN combine matmuls)
            consts = {}
            for nm in ("bqk", "gamma", "beta"):
                consts[nm] = cp.tile([128, list(w_in[nm].shape)[1]], F32, name=nm)
                nc.sync.dma_start(consts[nm][:], w_in[nm].ap())
            consts["gn_fwd"] = cp.tile([128, OC, 32], F32, name="gn_fwd")
            nc.sync.dma_start(consts["gn_fwd"][:], w_in["gn_fwd"].ap().rearrange("j p g -> p j g"))
            consts["gn_bwd"] = cp.tile([32, OC, 128], F32, name="gn_bwd")
            nc.sync.dma_start(consts["gn_bwd"][:], w_in["gn_bwd"].ap().rearrange("j g c -> g j c"))
            consts["eps"] = cp.tile([128, 1], F32, name="eps")
            nc.vector.memset(consts["eps"][:], EPS)
            consts["negtwo"] = cp.tile([128, 1], F32, name="negtwo")
            nc.vector.memset(consts["negtwo"][:], -2.0)

            consts["ones8"] = cp.tile([1, 512], F8, name="ones8")
            nc.vector.memset(consts["ones8"][:], 1.0)
            consts["beff"] = cp.tile([1, OC, 128], F8, name="beff")
            nc.gpsimd.dma_start(consts["beff"][:], w_in["beff"].ap())

            # packed AV lhsT: [c-part, t-chunk, head, 128] fp8.
            # even head: [v(64) | 1 | 0*63]; odd head: [1 | 0*63 | v(64)].
            # ones/zero lanes initialized once (v slots rewritten per iter).
            consts["av_lhs"] = cp.tile([128, TC, NH, 128], F8, name="av_lhs")
            av4 = consts["av_lhs"][:].rearrange("p t (pr two) m -> p t pr two m", two=2)
            nc.vector.memset(av4[:, :, :, 0, 64:65], 1.0)
            nc.vector.memset(av4[:, :, :, 0, 65:128], 0.0)
            nc.vector.memset(av4[:, :, :, 1, 0:1], 1.0)
            nc.vector.memset(av4[:, :, :, 1, 1:64], 0.0)

            x_pre = None
            if loop_iters == 1:
                x_pre = emit_x_load(nc, pools, dram)

            # fp8 weights: gpsimd DMA casts f32 DRAM -> fp8 SBUF
            wqk_r = w_in["wqk"].ap().rearrange("k p h m -> p k h m")
            consts["wqk"] = cp.tile([128, KS, NH, 128], F8, name="wqk")
            consts["wvt"] = cp.tile([128, KS, 512], F8, name="wvt")
            consts["wpt"] = cp.tile([128, KS, 512], F8, name="wpt")
            for h2 in range(NH):
                nc.gpsimd.dma_start(consts["wqk"][:, :, h2:h2 + 1, :], wqk_r[:, :, h2:h2 + 1, :])
                if h2 == 1:
                    nc.gpsimd.dma_start(consts["wvt"][:], w_in["wvt"].ap().rearrange("k p n -> p k n"))
            nc.gpsimd.dma_start(consts["wpt"][:], w_in["wpt"].ap().rearrange("k p n -> p k n"))

            if loop_iters > 1:
                with tctx.For_i(0, loop_iters, 1, hint_engines=(mybir.EngineType.PE,)):
                    emit_body(nc, tctx, pools, dram, consts)
            else:
                emit_body(nc, tctx, pools, dram, consts, x_sb=x_pre)

    nc.compile()
    return nc


def prep_weights(gamma, beta, w_qkv, b_qkv, w_proj, b_proj):
    q_rows = np.concatenate([np.arange(192 * h, 192 * h + 64) for h in range(NH)])
    k_rows = q_rows + 64
    v_rows = q_rows + 128
    chunk_rows = []
    for p in range(NH // 2):
        chunk_rows.append(np.concatenate([q_rows[128 * p:128 * p + 64],
                                          q_rows[128 * p + 64:128 * p + 128]]))
        chunk_rows.append(np.concatenate([k_rows[128 * p:128 * p + 64],
                                          k_rows[128 * p + 64:128 * p + 128]]))
    wqk = np.stack([w_qkv[rows, :] for rows in chunk_rows])       # [8, 128, 512]
    tmp = wqk.transpose(2, 0, 1)          # [512(c), 8(h), 128(m)]
    wqk_t = np.ascontiguousarray(tmp.reshape(KS, 128, NH, 128))
    bqk = np.ascontiguousarray(np.stack([b_qkv[rows] for rows in chunk_rows], axis=1))

    wv = w_qkv[v_rows, :]
    wvt = np.ascontiguousarray(wv.T.reshape(KS, 128, 512))
    wpt = np.ascontiguousarray((w_proj * 512.0).T.reshape(KS, 128, 512))

    b_v = b_qkv[v_rows]
    b_eff = (b_proj.astype(np.float64) + w_proj.astype(np.float64) @ b_v.astype(np.float64)).astype(np.float32)
    beff = np.ascontiguousarray((b_eff * 512.0).reshape(1, OC, 128))
    gamma_t = np.ascontiguousarray(np.asarray(gamma, np.float32).reshape(OC, 128).T)
    beta_t = np.ascontiguousarray(np.asarray(beta, np.float32).reshape(OC, 128).T)

    gn_fwd = np.zeros((OC, 128, 32), np.float32)
    gn_bwd = np.zeros((OC, 32, 128), np.float32)
    for j in range(OC):
        for pp in range(128):
            gn_fwd[j, pp, (128 * j + pp) // 16] = 1.0 / 16.0
            gn_bwd[j, (128 * j + pp) // 16, pp] = 1.0
    return {"wqk": wqk_t, "wvt": wvt, "wpt": wpt, "bqk": bqk, "beff": beff,
            "gamma": gamma_t, "beta": beta_t, "gn_fwd": gn_fwd, "gn_bwd": gn_bwd}


_STATE = {}
N_CORES = 8


class _SpmdRunner:
    def __init__(self, nc, n_cores):
        import jax
        from jax.sharding import Mesh, PartitionSpec
        from jax.experimental.shard_map import shard_map
        from concourse.bass2jax import _bass_exec_p, partition_id_tensor, install_neuronx_cc_hook
        install_neuronx_cc_hook()
        self.n_cores = n_cores
        partition_name = nc.partition_id_tensor.name if nc.partition_id_tensor else None
        in_names, out_names, out_avals, zero_outs = [], [], [], []
        for alloc in nc.m.functions[0].allocations:
            if not isinstance(alloc, mybir.MemoryLocationSet):
                continue
            name = alloc.memorylocations[0].name
            if alloc.kind == "ExternalInput":
                if name != partition_name:
                    in_names.append(name)
            elif alloc.kind == "ExternalOutput":
                out_names.append(name)
                shape = tuple(alloc.tensor_shape)
                dtype = mybir.dt.np(alloc.dtype)
                out_avals.append(jax.core.ShapedArray(shape, dtype))
                zero_outs.append(np.zeros(shape, dtype))
        self.in_names, self.out_names = in_names, out_names
        self.out_avals, self.zero_outs = out_avals, zero_outs
        n_params, n_outs = len(in_names), len(out_avals)
        all_in_names = list(in_names) + list(out_names)
        if partition_name is not None:
            all_in_names.append(partition_name)

        def _body(*args):
            operands = list(args)
            if partition_name is not None:
                operands.append(partition_id_tensor())
            outs = _bass_exec_p.bind(
                *operands, out_avals=tuple(out_avals), in_names=tuple(all_in_names),
                out_names=tuple(out_names), lowering_input_output_aliases=(),
                sim_require_finite=True, sim_require_nnan=True, nc=nc)
            return tuple(outs)

        devices = jax.devices()[:n_cores]
        mesh = Mesh(np.asarray(devices), ("core",))
        in_specs = (PartitionSpec("core"),) * (n_params + n_outs)
        out_specs = (PartitionSpec("core"),) * n_outs
        self.sharded = jax.jit(
            shard_map(_body, mesh=mesh, in_specs=in_specs, out_specs=out_specs, check_rep=False),
            donate_argnums=tuple(range(n_params, n_params + n_outs)), keep_unused=True)

    def __call__(self, in_maps):
        n_cores = self.n_cores
        per_core = [[np.asarray(m[name]) for name in self.in_names] for m in in_maps]
        concat_in = [np.concatenate([per_core[c][i] for c in range(n_cores)], axis=0)
                     for i in range(len(self.in_names))]
        concat_zeros = [np.zeros((n_cores * z.shape[0], *z.shape[1:]), z.dtype)
                        for z in self.zero_outs]
        out_arrs = self.sharded(*concat_in, *concat_zeros)
        return [
            {name: np.asarray(out_arrs[i]).reshape(n_cores, *self.out_avals[i].shape)[c]
             for i, name in enumerate(self.out_names)}
            for c in range(n_cores)
        ]


def kernel(x, gamma, beta, w_qkv, b_qkv, w_proj, b_proj):
    x = np.asarray(x, np.float32)
    assert x.shape == (8, C, 32, 32), x.shape
    w = prep_weights(np.asarray(gamma, np.float32), np.asarray(beta, np.float32),
                     np.asarray(w_qkv, np.float32), np.asarray(b_qkv, np.float32),
                     np.asarray(w_proj, np.float32), np.asarray(b_proj, np.float32))
    if "runner" not in _STATE:
        nc = build_nc(loop_iters=1)
        _STATE["runner"] = _SpmdRunner(nc, N_CORES)
    in_maps = []
    for b in range(N_CORES):
        m = {"x": np.ascontiguousarray(x[b].reshape(C, S))}
        m.update(w)
        in_maps.append(m)
    res = _STATE["runner"](in_maps)
    out = np.stack([res[b]["y"] for b in range(N_CORES)]).reshape(8, C, 32, 32)
    return out.astype(np.float32)
